# revision 4
# baseline (speedup 1.0000x reference)
"""Trainium2 Bass kernel for nn_MultiHeadAttention (B=2, S=2048, D=1024, H=16).

Sharding: batch*heads across 8 cores -> each core handles one batch element's
4 heads (core c: b = c//4, heads h0 = (c%4)*4 .. h0+4).

Key idea: the padding mask kills ~half the keys; the host gathers each head's
unmasked key positions (padded to KT tiles of 128) so scores/exp/ctx run over
~9 instead of 16 key tiles.

v2 structure (fp16 matmuls, f32 PSUM):
  - DMA: few large sprayed transfers, ordered so the PE never starves
    (consts, Wq, xT, xg0, Wk, Wv, xg1-3, Wo).
  - Q projection into two zero-padded transposed tiles (memzero, not DMA).
  - K projection via the pair-discard trick (unchanged).
  - V projection flipped: Wv is the stationary operand, xg streams 512-wide;
    bias folded into the PSUM drain (per-partition scalar); PE transpose
    produces v4 [key, vdim] tiles.
  - Attention: one flat 36-step software pipeline across the 4 (pair, half)
    blocks; score matmuls ordered s0-qc0, s0-qc1, s1-qc0, s1-qc1 so the
    s0 PSUM slot refills immediately after exp(hp0) frees it.  Target
    steady-state period = 2 ScalarE exps = ~2.0us per step.
  - Normalization: 1/rowsum broadcast via gpsimd.partition_broadcast into
    SBUF (no PE matmul, no PSUM slot), DVE multiply into ctxT.
  - Output projection entirely in the tail, reading per-half ctxT tiles so
    coarse tile deps don't serialize qt 0-7 behind the last block's norms.
Host sums the 4 partial outputs per batch element and adds b_out.
"""

import math
import os

import numpy as np

# Tile's fine-grained (subtile) dependency tracker misses some of this
# kernel's partition-sliced producer->consumer edges (verified empirically:
# per-core divergent results with it on, bit-identical and correct with it
# off). Coarse tile-level deps cost little here and are always safe.
os.environ.setdefault("BY_DEFAULT_DISABLE_SUBTILE_DEPS", "1")

N_HEADS = 16
DIM = 1024
DIM_PER_HEAD = 64
B = 2
S = 2048
SCALE = math.sqrt(DIM_PER_HEAD)
N_CORES = 8
HEADS_PER_CORE = 4

_cache = {}


def _build_program(KT):
    import concourse.tile as tile
    from concourse import bacc, masks, mybir

    f32 = mybir.dt.float32
    fp16 = mybir.dt.float16
    Exp = mybir.ActivationFunctionType.Exp
    SK = KT * 128  # gathered (padded) key count per head

    nc = bacc.Bacc("TRN2", target_bir_lowering=False, debug=False,
                   num_devices=N_CORES)

    xT = nc.dram_tensor("xT", [DIM, S], fp16, kind="ExternalInput").ap()
    xg = nc.dram_tensor("xg", [4, DIM, SK], fp16, kind="ExternalInput").ap()
    Wq = nc.dram_tensor("Wq", [DIM, 256], fp16, kind="ExternalInput").ap()
    Wk = nc.dram_tensor("Wk", [DIM, 256], fp16, kind="ExternalInput").ap()
    Wv = nc.dram_tensor("Wv", [DIM, 256], fp16, kind="ExternalInput").ap()
    Wo = nc.dram_tensor("Wo", [256, DIM], fp16, kind="ExternalInput").ap()
    bqk = nc.dram_tensor("bqk", [128, 4], f32, kind="ExternalInput").ap()
    bvT = nc.dram_tensor("bvT", [64, 4], f32, kind="ExternalInput").ap()
    maskT = nc.dram_tensor("maskT", [128, 4 * KT], f32,
                           kind="ExternalInput").ap()
    out_d = nc.dram_tensor("out", [S, DIM], fp16, kind="ExternalOutput").ap()

    with tile.TileContext(nc) as tc:
        with tc.tile_pool(name="const", bufs=1) as cpool, \
             tc.tile_pool(name="wpool", bufs=1) as wpool, \
             tc.tile_pool(name="xgp", bufs=1) as xgp, \
             tc.tile_pool(name="qkv", bufs=1) as qkvp, \
             tc.tile_pool(name="ps", bufs=2, space="PSUM") as ps:

            # ---- input DMAs, ordered by first use; each is one big sprayed
            # transfer so the queues run at HBM roofline ----
            maskT_sb = cpool.tile([128, 4 * KT], f32)
            nc.sync.dma_start(maskT_sb[:], maskT[:])
            bqk_sb = cpool.tile([128, 4], f32)
            nc.sync.dma_start(bqk_sb[:], bqk[:])
            bvT_sb = cpool.tile([64, 4], f32)
            nc.sync.dma_start(bvT_sb[:], bvT[:])
            Wq_sb = wpool.tile([128, 8, 256], fp16)
            nc.sync.dma_start(Wq_sb[:], Wq.rearrange("(c p) j -> p c j", p=128))

            # identity for PE transposes (device-built, no DMA)
            id_sb = cpool.tile([64, 64], fp16)
            masks.make_identity(nc, id_sb[:])

            # Q/K/V targets
            Qt0 = qkvp.tile([128, 2, S], fp16)
            Qt1 = qkvp.tile([128, 2, S], fp16)
            nc.gpsimd.memset(Qt0[64:128, :, :], 0.0)
            nc.gpsimd.memset(Qt1[0:64, :, :], 0.0)
            Kt_p = [qkvp.tile([128, SK], fp16, name=f"Kt_{p}")
                    for p in range(2)]
            v4_h = [qkvp.tile([128, KT, 65], fp16, name=f"v4_{hl}")
                    for hl in range(4)]
            for hl in range(4):
                nc.gpsimd.memset(v4_h[hl][:, :, 64], 1.0)
            ctxT_h = [qkvp.tile([128, 2, 1024], fp16, name=f"ctxT_{half}")
                      for half in range(2)]

            with tc.tile_pool(name="xsub", bufs=1) as xsub:
                xts = xsub.tile([128, 8, S], fp16)
                nc.sync.dma_start(xts[:],
                                  xT.rearrange("(c p) s -> p c s", p=128))

                xg_t = []
                for hl in range(4):
                    t = xgp.tile([128, 8, SK], fp16, tag="xg",
                                 name=f"xg_{hl}")
                    nc.sync.dma_start(
                        t[:], xg[hl].rearrange("(c p) k -> p c k", p=128))
                    xg_t.append(t)
                    if hl == 0:
                        Wk_sb = wpool.tile([128, 8, 256], fp16)
                        nc.sync.dma_start(
                            Wk_sb[:], Wk.rearrange("(c p) j -> p c j", p=128))
                        Wv_sb = wpool.tile([128, 8, 256], fp16)
                        nc.sync.dma_start(
                            Wv_sb[:], Wv.rearrange("(c p) j -> p c j", p=128))
                Wo_sb = wpool.tile([128, 2, 1024], fp16)
                nc.sync.dma_start(Wo_sb[:],
                                  Wo.rearrange("(c p) e -> p c e", p=128))

                # ---- Q projection (transposed, zero-padded per head) ----
                for sc in range(4):
                    for p in range(2):
                        ps_t = ps.tile([128, 512], f32,
                                       tag="a" if p == 0 else "ctx",
                                       name=f"pq_{sc}_{p}")
                        for dc in range(8):
                            nc.tensor.matmul(
                                ps_t[:],
                                lhsT=Wq_sb[:, dc, p * 128:(p + 1) * 128],
                                rhs=xts[:, dc, sc * 512:(sc + 1) * 512],
                                start=(dc == 0), stop=(dc == 7))
                        ssl = slice(sc * 512, (sc + 1) * 512)
                        bias = bqk_sb[:, p: p + 1]
                        nc.vector.tensor_scalar_add(
                            Qt0[0:64, p, ssl], ps_t[0:64, :], bias[0:64, :])
                        nc.vector.tensor_scalar_add(
                            Qt1[64:128, p, ssl], ps_t[64:128, :],
                            bias[64:128, :])

            with tc.tile_pool(name="vtp", bufs=2) as vtp, \
                 tc.tile_pool(name="expp", bufs=5) as expp, \
                 tc.tile_pool(name="ctxu", bufs=2) as ctxu, \
                 tc.tile_pool(name="bcp", bufs=4) as bcp, \
                 tc.tile_pool(name="outsb", bufs=4) as outsb, \
                 tc.tile_pool(name="rscr", bufs=2) as rscr:

                nchunks = []
                n0 = 0
                while n0 < SK:
                    nn = min(512, SK - n0)
                    nchunks.append((n0, nn))
                    n0 += nn

                # ---- K and V projection, per head in DMA-arrival order ----
                for hl in range(4):
                    p, hp = hl // 2, hl % 2
                    # K: pair-discard trick -> Kt_p[p] rows hp*64:(hp+1)*64
                    for ci, (c0, nn) in enumerate(nchunks):
                        ps_t = ps.tile([128, 512], f32,
                                       tag="a" if ci % 2 == 0 else "ctx",
                                       name=f"pk_{hl}_{ci}")
                        for dc in range(8):
                            nc.tensor.matmul(
                                ps_t[:, 0:nn],
                                lhsT=Wk_sb[:, dc, p * 128:(p + 1) * 128],
                                rhs=xg_t[hl][:, dc, c0:c0 + nn],
                                start=(dc == 0), stop=(dc == 7))
                        bias = bqk_sb[:, 2 + p: 3 + p]
                        nc.vector.tensor_scalar_add(
                            Kt_p[p][hp * 64:(hp + 1) * 64, c0:c0 + nn],
                            ps_t[hp * 64:(hp + 1) * 64, 0:nn],
                            bias[hp * 64:(hp + 1) * 64, :])

                    # V flipped: out VT [64 vdim, keys]; Wv slice stationary
                    pv = []
                    for ci, (c0, nn) in enumerate(nchunks):
                        pv.append(ps.tile([128, nn], f32,
                                          tag="a" if ci % 2 == 0 else "ctx",
                                          name=f"pv_{hl}_{ci}"))
                    for dc in range(8):
                        for ci, (c0, nn) in enumerate(nchunks):
                            nc.tensor.matmul(
                                pv[ci][0:64, :],
                                lhsT=Wv_sb[:, dc, hl * 64:(hl + 1) * 64],
                                rhs=xg_t[hl][:, dc, c0:c0 + nn],
                                start=(dc == 0), stop=(dc == 7))
                    VT = vtp.tile([64, SK], fp16, tag="vt", name=f"vt_{hl}")
                    for ci, (c0, nn) in enumerate(nchunks):
                        nc.vector.tensor_scalar_add(
                            VT[:, c0:c0 + nn], pv[ci][0:64, :],
                            bvT_sb[:, hl:hl + 1])
                    # transpose VT 128-key chunks into v4 [key, vdim]
                    for kt in range(KT):
                        pt = ps.tile([128, 64], fp16, tag="ctx",
                                     name=f"pt_{hl}_{kt}")
                        nc.tensor.transpose(
                            pt[:], VT[:, kt * 128:(kt + 1) * 128], id_sb[:])
                        nc.vector.tensor_copy(v4_h[hl][:, kt, 0:64], pt[:])

                # ---- attention: flat 36-step pipeline over 4 blocks ----
                # block b: p = b//2, half = b%2 (pair-major: pair 0 is ready
                # first). Step t: block(t) = t//KT, kt(t) = t%KT.
                NSTEP = 4 * KT

                def blk(t):
                    return (t // KT) // 2, (t // KT) % 2, t % KT

                def emit_scores(t):
                    p, half, kt = blk(t)
                    s0 = ps.tile([128, 1024], f32, tag="a",
                                 name=f"s0_{t}")
                    s1 = ps.tile([128, 1024], f32, tag="a",
                                 name=f"s1_{t}")
                    lhsT = Kt_p[p][:, kt * 128:(kt + 1) * 128]
                    for s_t, qsrc in ((s0, Qt0), (s1, Qt1)):
                        for qc in range(2):
                            q0 = half * 1024 + qc * 512
                            nc.tensor.matmul(
                                s_t[:, qc * 512:(qc + 1) * 512],
                                lhsT=lhsT,
                                rhs=qsrc[:, p, q0:q0 + 512],
                                start=True, stop=True)
                    return s0, s1

                def emit_exp(t, sc_t):
                    p, half, kt = blk(t)
                    ets = []
                    for hp in range(2):
                        et = expp.tile([128, 1024], fp16, tag="et",
                                       name=f"et_{t}_{hp}")
                        nc.scalar.activation(
                            et[:], sc_t[hp][:], Exp,
                            bias=maskT_sb[:, kt * 4 + 2 * p + hp:
                                          kt * 4 + 2 * p + hp + 1],
                            scale=1.0)
                        ets.append(et)
                    return ets

                ctxs_of_block = {}

                def emit_ctx(t, ets):
                    p, half, kt = blk(t)
                    b = t // KT
                    if kt == 0:
                        ctxs_of_block[b] = (
                            ps.tile([65, 1024], f32, tag="ctx", name=f"c0_{b}"),
                            ps.tile([65, 1024], f32, tag="ctx", name=f"c1_{b}"))
                    ctxs = ctxs_of_block[b]
                    for hp in range(2):
                        for qc in range(2):
                            nc.tensor.matmul(
                                ctxs[hp][:, qc * 512:(qc + 1) * 512],
                                lhsT=v4_h[2 * p + hp][:, kt, :],
                                rhs=ets[hp][:, qc * 512:(qc + 1) * 512],
                                start=(kt == 0), stop=(kt == KT - 1))

                norm_q = []

                def emit_drain(b):
                    # block b finished accumulating: move ctx out of PSUM,
                    # compute 1/rowsum, broadcast it across partitions on
                    # gpsimd, queue the normalize multiplies.
                    p, half = b // 2, b % 2
                    ctxs = ctxs_of_block.pop(b)
                    ctxUs, bcs = [], []
                    for hp in range(2):
                        ctxU = ctxu.tile([65, 1024], fp16, tag="cu", bufs=4,
                                         name=f"cu_{b}_{hp}")
                        nc.vector.tensor_copy(ctxU[:], ctxs[hp][:])
                        s128 = rscr.tile([128, 8], fp16, tag="sm",
                                         name=f"sm_{b}_{hp}")
                        nc.sync.dma_start(s128[:], ctxU[64:65, :])
                        r128 = rscr.tile([128, 8], fp16, tag="rc",
                                         name=f"rc_{b}_{hp}")
                        with nc.allow_low_precision(
                                reason="fp16 softmax-sum reciprocal"):
                            nc.vector.reciprocal(r128[:], s128[:])
                        rs_t = rscr.tile([1, 1024], fp16, tag="rs",
                                         name=f"rs_{b}_{hp}")
                        nc.sync.dma_start(rs_t[:], r128[:])
                        bc_t = bcp.tile([64, 1024], fp16, tag="bc", bufs=4,
                                        name=f"bc_{b}_{hp}")
                        nc.gpsimd.partition_broadcast(bc_t[:], rs_t[0:1, :])
                        ctxUs.append(ctxU)
                        bcs.append(bc_t)

                    box = {}

                    def step(j):
                        hp_, qc = j // 2, j % 2
                        if hp_ == 0:
                            tgt = ctxT_h[half][0:64, p,
                                              qc * 512:(qc + 1) * 512]
                        else:
                            if "t" not in box:
                                box["t"] = ctxu.tile([64, 1024], fp16,
                                                     tag="cn", bufs=2,
                                                     name=f"cn_{b}")
                            tgt = box["t"][0:64, qc * 512:(qc + 1) * 512]
                        nc.vector.tensor_mul(
                            tgt, ctxUs[hp_][0:64, qc * 512:(qc + 1) * 512],
                            bcs[hp_][0:64, qc * 512:(qc + 1) * 512])
                        if hp_ == 1 and qc == 1:
                            nc.sync.dma_start(
                                ctxT_h[half][64:128, p, :], box["t"][0:64, :])

                    norm_q.extend([lambda j=j: step(j) for j in range(4)])

                sc_cur = emit_scores(0)
                prev = None  # (t-1, ets)
                for t in range(NSTEP):
                    ets = emit_exp(t, sc_cur)
                    if prev is not None:
                        emit_ctx(prev[0], prev[1])
                        if prev[0] % KT == KT - 1:
                            emit_drain(prev[0] // KT)
                    if t < NSTEP - 1:
                        sc_cur = emit_scores(t + 1)
                    prev = (t, ets)
                    if norm_q and 2 <= (t % KT) <= 5:
                        norm_q.pop(0)()
                emit_ctx(prev[0], prev[1])
                emit_drain(3)

                # ---- output projection tail ----
                for st_fn in norm_q:
                    st_fn()
                for qt in range(16):
                    half, c = qt // 8, qt % 8
                    tag = "a" if qt % 2 == 0 else "ctx"
                    po = ps.tile([128, 1024], f32, tag=tag, name=f"po_{qt}")
                    for p_ in range(2):
                        for ec in range(2):
                            nc.tensor.matmul(
                                po[:, ec * 512:(ec + 1) * 512],
                                lhsT=ctxT_h[half][:, p_,
                                                  c * 128:(c + 1) * 128],
                                rhs=Wo_sb[:, p_, ec * 512:(ec + 1) * 512],
                                start=(p_ == 0), stop=(p_ == 1))
                    ob = outsb.tile([128, 1024], fp16, tag="ob",
                                    name=f"ob_{qt}")
                    if qt % 2 == 0:
                        nc.vector.tensor_copy(ob[:], po[:])
                    else:
                        nc.scalar.copy(ob[:], po[:])
                    nc.sync.dma_start(out_d[qt * 128:(qt + 1) * 128, :],
                                      ob[:])

    nc.compile()
    return nc


def get_program(KT=9):
    key = ("nc", KT)
    if key not in _cache:
        _cache[key] = _build_program(KT)
    return _cache[key]


def make_in_maps(query, mask, W_qkv, b_qkv, W_out, b_out):
    query = np.asarray(query, dtype=np.float32)
    mask = np.asarray(mask)
    W_qkv = np.asarray(W_qkv, dtype=np.float32)
    b_qkv = np.asarray(b_qkv, dtype=np.float32)
    W_out = np.asarray(W_out, dtype=np.float32)
    bf = np.float16

    W3 = W_qkv.reshape(DIM, N_HEADS, DIM_PER_HEAD, 3)
    b3 = b_qkv.reshape(N_HEADS, DIM_PER_HEAD, 3)
    m2 = np.asarray(mask)[:, 0, :]  # [32, 2048] True = masked
    KT = max(1, int(np.ceil((~m2).sum(axis=1).max() / 128)))
    SK = KT * 128

    in_maps = []
    for c in range(N_CORES):
        b = c // 4
        h0 = (c % 4) * HEADS_PER_CORE
        hs = slice(h0, h0 + HEADS_PER_CORE)
        Wq_c = np.ascontiguousarray(
            W3[:, hs, :, 0].reshape(DIM, 256) / SCALE).astype(bf)
        Wk_c = np.ascontiguousarray(W3[:, hs, :, 1].reshape(DIM, 256)).astype(bf)
        Wv_c = np.ascontiguousarray(W3[:, hs, :, 2].reshape(DIM, 256)).astype(bf)
        bq_c = (b3[hs, :, 0].reshape(256) / SCALE).astype(np.float32)
        bk_c = b3[hs, :, 1].reshape(256).astype(np.float32)
        bvT_c = np.ascontiguousarray(b3[hs, :, 2].T).astype(np.float32)  # [64, 4]
        bqk_c = np.ascontiguousarray(
            np.stack([bq_c[:128], bq_c[128:], bk_c[:128], bk_c[128:]], axis=1))
        Wo_c = np.ascontiguousarray(
            W_out[h0 * 64:(h0 + 4) * 64, :]).astype(bf)
        xT_c = np.ascontiguousarray(query[b].T).astype(bf)

        xg_c = np.zeros((4, DIM, SK), dtype=bf)
        maskT_c = np.zeros((128, 4 * KT), dtype=np.float32)
        for hl in range(4):
            bh = b * N_HEADS + h0 + hl
            idx = np.nonzero(~m2[bh])[0]
            n = len(idx)
            idx_pad = np.zeros(SK, dtype=np.int64)
            idx_pad[:n] = idx
            xg_c[hl] = xT_c[:, idx_pad]
            padded = np.arange(SK) >= n  # [SK] True = padding slot
            maskT_c[:, hl::4] = np.where(
                padded.reshape(KT, 128).T, np.float32(-30000.0),
                np.float32(0.0))
        in_maps.append({
            "xT": xT_c, "xg": xg_c, "Wq": Wq_c, "Wk": Wk_c, "Wv": Wv_c,
            "Wo": Wo_c, "bqk": bqk_c, "bvT": bvT_c, "maskT": maskT_c,
        })
    return in_maps, KT


def gather_outputs(results, b_out):
    b_out = np.asarray(b_out, dtype=np.float32)
    out = np.zeros((B, S, DIM), dtype=np.float32)
    for c in range(N_CORES):
        out[c // 4] += results[c]["out"].astype(np.float32)
    out += b_out[None, None, :]
    return out


def kernel(query, mask, W_qkv, b_qkv, W_out, b_out):
    from concourse.bass_utils import run_bass_kernel_spmd

    in_maps, KT = make_in_maps(query, mask, W_qkv, b_qkv, W_out, b_out)
    nc = get_program(KT)
    res = run_bass_kernel_spmd(nc, in_maps, list(range(N_CORES)))
    return gather_outputs(res.results, b_out)


# revision 6
# speedup vs baseline: 1.0693x; 1.0693x over previous
"""Trainium2 Bass kernel for nn_MultiHeadAttention (B=2, S=2048, D=1024, H=16).

Sharding: batch*heads across 8 cores -> each core handles one batch element's
4 heads (core c: b = c//4, heads h0 = (c%4)*4 .. h0+4).

Key idea: the padding mask kills ~half the keys; the host gathers each head's
unmasked key positions (padded to KT tiles of 128) so scores/exp/ctx run over
~9 instead of 16 key tiles.

v2 structure (fp16 matmuls, f32 PSUM):
  - DMA: few large sprayed transfers, ordered so the PE never starves
    (consts, Wq, xT, xg0, Wk, Wv, xg1-3, Wo).
  - Q projection into two zero-padded transposed tiles (memzero, not DMA).
  - K projection via the pair-discard trick (unchanged).
  - V projection flipped: Wv is the stationary operand, xg streams 512-wide;
    bias folded into the PSUM drain (per-partition scalar); PE transpose
    produces v4 [key, vdim] tiles.
  - Attention: one flat 36-step software pipeline across the 4 (pair, half)
    blocks; score matmuls ordered s0-qc0, s0-qc1, s1-qc0, s1-qc1 so the
    s0 PSUM slot refills immediately after exp(hp0) frees it.  Target
    steady-state period = 2 ScalarE exps = ~2.0us per step.
  - Normalization: 1/rowsum broadcast via gpsimd.partition_broadcast into
    SBUF (no PE matmul, no PSUM slot), DVE multiply into ctxT.
  - Output projection entirely in the tail, reading per-half ctxT tiles so
    coarse tile deps don't serialize qt 0-7 behind the last block's norms.
Host sums the 4 partial outputs per batch element and adds b_out.
"""

import math
import os

import numpy as np

# Tile's fine-grained (subtile) dependency tracker misses some of this
# kernel's partition-sliced producer->consumer edges (verified empirically:
# per-core divergent results with it on, bit-identical and correct with it
# off). Coarse tile-level deps cost little here and are always safe.
os.environ.setdefault("BY_DEFAULT_DISABLE_SUBTILE_DEPS", "1")

N_HEADS = 16
DIM = 1024
DIM_PER_HEAD = 64
B = 2
S = 2048
SCALE = math.sqrt(DIM_PER_HEAD)
N_CORES = 8
HEADS_PER_CORE = 4

_cache = {}


def _build_program(KT):
    import concourse.tile as tile
    from concourse import bacc, masks, mybir

    f32 = mybir.dt.float32
    fp16 = mybir.dt.float16
    Exp = mybir.ActivationFunctionType.Exp
    SK = KT * 128  # gathered (padded) key count per head

    nc = bacc.Bacc("TRN2", target_bir_lowering=False, debug=False,
                   num_devices=N_CORES)

    xT = nc.dram_tensor("xT", [DIM, S], fp16, kind="ExternalInput").ap()
    xg = nc.dram_tensor("xg", [4, DIM, SK], fp16, kind="ExternalInput").ap()
    Wq = nc.dram_tensor("Wq", [DIM, 256], fp16, kind="ExternalInput").ap()
    Wk = nc.dram_tensor("Wk", [DIM, 256], fp16, kind="ExternalInput").ap()
    Wv = nc.dram_tensor("Wv", [DIM, 256], fp16, kind="ExternalInput").ap()
    Wo = nc.dram_tensor("Wo", [256, DIM], fp16, kind="ExternalInput").ap()
    bqk = nc.dram_tensor("bqk", [128, 4], f32, kind="ExternalInput").ap()
    bvT = nc.dram_tensor("bvT", [64, 4], f32, kind="ExternalInput").ap()
    maskT = nc.dram_tensor("maskT", [128, 4 * KT], f32,
                           kind="ExternalInput").ap()
    out_d = nc.dram_tensor("out", [S, DIM], fp16, kind="ExternalOutput").ap()

    with tile.TileContext(nc) as tc:
        with tc.tile_pool(name="const", bufs=1) as cpool, \
             tc.tile_pool(name="wpool", bufs=1) as wpool, \
             tc.tile_pool(name="xgp", bufs=1) as xgp, \
             tc.tile_pool(name="qkv", bufs=1) as qkvp, \
             tc.tile_pool(name="ps", bufs=2, space="PSUM") as ps:

            # ---- input DMAs, ordered by first use; each is one big sprayed
            # transfer so the queues run at HBM roofline ----
            maskT_sb = cpool.tile([128, 4 * KT], f32)
            nc.sync.dma_start(maskT_sb[:], maskT[:])
            bqk_sb = cpool.tile([128, 4], f32)
            nc.sync.dma_start(bqk_sb[:], bqk[:])
            bvT_sb = cpool.tile([64, 4], f32)
            nc.sync.dma_start(bvT_sb[:], bvT[:])
            Wq_sb = wpool.tile([128, 8, 256], fp16)
            nc.sync.dma_start(Wq_sb[:], Wq.rearrange("(c p) j -> p c j", p=128))

            # identity for PE transposes (device-built, no DMA)
            id_sb = cpool.tile([64, 64], fp16)
            masks.make_identity(nc, id_sb[:])

            # Q/K/V targets
            Qt0 = qkvp.tile([128, 2, S], fp16)
            Qt1 = qkvp.tile([128, 2, S], fp16)
            nc.gpsimd.memset(Qt0[64:128, :, :], 0.0)
            nc.gpsimd.memset(Qt1[0:64, :, :], 0.0)
            Kt_p = [qkvp.tile([128, SK], fp16, name=f"Kt_{p}")
                    for p in range(2)]
            v4_h = [qkvp.tile([128, KT, 65], fp16, name=f"v4_{hl}")
                    for hl in range(4)]
            for hl in range(4):
                nc.gpsimd.memset(v4_h[hl][:, :, 64], 1.0)
            ctxT_h = [qkvp.tile([128, 2, 1024], fp16, name=f"ctxT_{half}")
                      for half in range(2)]

            with tc.tile_pool(name="xsub", bufs=1) as xsub:
                # xT in 4 query-chunk tiles so Q proj starts after ~1MB;
                # xg interleaved so each head's gather lands just in time.
                xT_r = xT.rearrange("(c p) s -> p c s", p=128)
                xts = []
                for sc in range(4):
                    t = xsub.tile([128, 8, 512], fp16, name=f"xts_{sc}")
                    nc.sync.dma_start(t[:],
                                      xT_r[:, :, sc * 512:(sc + 1) * 512])
                    xts.append(t)
                    if sc == 1:
                        xg0 = xgp.tile([128, 8, SK], fp16, name="xg_0")
                        nc.sync.dma_start(
                            xg0[:], xg[0].rearrange("(c p) k -> p c k", p=128))
                Wk_sb = wpool.tile([128, 8, 256], fp16)
                nc.sync.dma_start(
                    Wk_sb[:], Wk.rearrange("(c p) j -> p c j", p=128))
                Wv_sb = wpool.tile([128, 8, 256], fp16)
                nc.sync.dma_start(
                    Wv_sb[:], Wv.rearrange("(c p) j -> p c j", p=128))
                xg_t = [xg0]
                for hl in range(1, 4):
                    t = xgp.tile([128, 8, SK], fp16, name=f"xg_{hl}")
                    nc.sync.dma_start(
                        t[:], xg[hl].rearrange("(c p) k -> p c k", p=128))
                    xg_t.append(t)
                Wo_sb = wpool.tile([128, 2, 1024], fp16)
                nc.sync.dma_start(Wo_sb[:],
                                  Wo.rearrange("(c p) e -> p c e", p=128))

                # ---- Q projection (transposed, zero-padded per head) ----
                for sc in range(4):
                    for p in range(2):
                        ps_t = ps.tile([128, 512], f32,
                                       tag="a" if p == 0 else "ctx",
                                       name=f"pq_{sc}_{p}")
                        for dc in range(8):
                            nc.tensor.matmul(
                                ps_t[:],
                                lhsT=Wq_sb[:, dc, p * 128:(p + 1) * 128],
                                rhs=xts[sc][:, dc, :],
                                start=(dc == 0), stop=(dc == 7))
                        ssl = slice(sc * 512, (sc + 1) * 512)
                        bias = bqk_sb[:, p: p + 1]
                        nc.vector.tensor_scalar_add(
                            Qt0[0:64, p, ssl], ps_t[0:64, :], bias[0:64, :])
                        nc.vector.tensor_scalar_add(
                            Qt1[64:128, p, ssl], ps_t[64:128, :],
                            bias[64:128, :])

            with tc.tile_pool(name="vtp", bufs=2) as vtp, \
                 tc.tile_pool(name="expp", bufs=5) as expp, \
                 tc.tile_pool(name="ctxu", bufs=2) as ctxu, \
                 tc.tile_pool(name="bcp", bufs=4) as bcp, \
                 tc.tile_pool(name="outsb", bufs=4) as outsb, \
                 tc.tile_pool(name="rscr", bufs=2) as rscr:

                nchunks = []
                n0 = 0
                while n0 < SK:
                    nn = min(512, SK - n0)
                    nchunks.append((n0, nn))
                    n0 += nn

                # ---- K and V projection, per head in DMA-arrival order ----
                for hl in range(4):
                    p, hp = hl // 2, hl % 2
                    # K: pair-discard trick -> Kt_p[p] rows hp*64:(hp+1)*64
                    for ci, (c0, nn) in enumerate(nchunks):
                        ps_t = ps.tile([128, 512], f32,
                                       tag="a" if ci % 2 == 0 else "ctx",
                                       name=f"pk_{hl}_{ci}")
                        for dc in range(8):
                            nc.tensor.matmul(
                                ps_t[:, 0:nn],
                                lhsT=Wk_sb[:, dc, p * 128:(p + 1) * 128],
                                rhs=xg_t[hl][:, dc, c0:c0 + nn],
                                start=(dc == 0), stop=(dc == 7))
                        bias = bqk_sb[:, 2 + p: 3 + p]
                        nc.vector.tensor_scalar_add(
                            Kt_p[p][hp * 64:(hp + 1) * 64, c0:c0 + nn],
                            ps_t[hp * 64:(hp + 1) * 64, 0:nn],
                            bias[hp * 64:(hp + 1) * 64, :])

                    # V flipped: out VT [64 vdim, keys]; Wv slice stationary
                    pv = []
                    for ci, (c0, nn) in enumerate(nchunks):
                        pv.append(ps.tile([128, nn], f32,
                                          tag="a" if ci % 2 == 0 else "ctx",
                                          name=f"pv_{hl}_{ci}"))
                    for dc in range(8):
                        for ci, (c0, nn) in enumerate(nchunks):
                            nc.tensor.matmul(
                                pv[ci][0:64, :],
                                lhsT=Wv_sb[:, dc, hl * 64:(hl + 1) * 64],
                                rhs=xg_t[hl][:, dc, c0:c0 + nn],
                                start=(dc == 0), stop=(dc == 7))
                    VT = vtp.tile([64, SK], fp16, tag="vt", name=f"vt_{hl}")
                    for ci, (c0, nn) in enumerate(nchunks):
                        nc.vector.tensor_scalar_add(
                            VT[:, c0:c0 + nn], pv[ci][0:64, :],
                            bvT_sb[:, hl:hl + 1])
                    # transpose VT 128-key chunks into v4 [key, vdim]
                    for kt in range(KT):
                        pt = ps.tile([128, 64], fp16, tag="ctx",
                                     name=f"pt_{hl}_{kt}")
                        nc.tensor.transpose(
                            pt[:], VT[:, kt * 128:(kt + 1) * 128], id_sb[:])
                        nc.vector.tensor_copy(v4_h[hl][:, kt, 0:64], pt[:])

                # ---- attention: flat 36-step pipeline over 4 blocks ----
                # block b: p = b//2, half = b%2 (pair-major: pair 0 is ready
                # first). Step t: block(t) = t//KT, kt(t) = t%KT.
                NSTEP = 4 * KT

                def blk(t):
                    return (t // KT) // 2, (t // KT) % 2, t % KT

                def emit_scores(t):
                    p, half, kt = blk(t)
                    s0 = ps.tile([128, 1024], f32, tag="a",
                                 name=f"s0_{t}")
                    s1 = ps.tile([128, 1024], f32, tag="a",
                                 name=f"s1_{t}")
                    lhsT = Kt_p[p][:, kt * 128:(kt + 1) * 128]
                    for s_t, qsrc in ((s0, Qt0), (s1, Qt1)):
                        for qc in range(2):
                            q0 = half * 1024 + qc * 512
                            nc.tensor.matmul(
                                s_t[:, qc * 512:(qc + 1) * 512],
                                lhsT=lhsT,
                                rhs=qsrc[:, p, q0:q0 + 512],
                                start=True, stop=True)
                    return s0, s1

                def emit_exp(t, sc_t):
                    p, half, kt = blk(t)
                    ets = []
                    for hp in range(2):
                        et = expp.tile([128, 1024], fp16, tag="et",
                                       name=f"et_{t}_{hp}")
                        nc.scalar.activation(
                            et[:], sc_t[hp][:], Exp,
                            bias=maskT_sb[:, kt * 4 + 2 * p + hp:
                                          kt * 4 + 2 * p + hp + 1],
                            scale=1.0)
                        ets.append(et)
                    return ets

                ctxs_of_block = {}

                def emit_ctx(t, ets):
                    p, half, kt = blk(t)
                    b = t // KT
                    if kt == 0:
                        ctxs_of_block[b] = (
                            ps.tile([65, 1024], f32, tag="ctx", name=f"c0_{b}"),
                            ps.tile([65, 1024], f32, tag="ctx", name=f"c1_{b}"))
                    ctxs = ctxs_of_block[b]
                    for hp in range(2):
                        for qc in range(2):
                            nc.tensor.matmul(
                                ctxs[hp][:, qc * 512:(qc + 1) * 512],
                                lhsT=v4_h[2 * p + hp][:, kt, :],
                                rhs=ets[hp][:, qc * 512:(qc + 1) * 512],
                                start=(kt == 0), stop=(kt == KT - 1))

                norm_q = []

                def emit_drain(b):
                    # block b finished accumulating: move ctx out of PSUM,
                    # compute 1/rowsum, broadcast it across partitions on
                    # gpsimd, queue the normalize multiplies.
                    p, half = b // 2, b % 2
                    ctxs = ctxs_of_block.pop(b)
                    ctxUs, bcs = [], []
                    for hp in range(2):
                        ctxU = ctxu.tile([65, 1024], fp16, tag="cu", bufs=4,
                                         name=f"cu_{b}_{hp}")
                        nc.vector.tensor_copy(ctxU[:], ctxs[hp][:])
                        rs_t = rscr.tile([1, 1024], fp16, tag="rs",
                                         name=f"rs_{b}_{hp}")
                        with nc.allow_low_precision(
                                reason="fp16 softmax-sum reciprocal"):
                            nc.vector.reciprocal(rs_t[:], ctxU[64:65, :])
                        bc_t = bcp.tile([64, 1024], fp16, tag="bc", bufs=4,
                                        name=f"bc_{b}_{hp}")
                        nc.gpsimd.partition_broadcast(bc_t[:], rs_t[0:1, :])
                        ctxUs.append(ctxU)
                        bcs.append(bc_t)

                    box = {}

                    def step(j):
                        hp_, qc = j // 2, j % 2
                        if hp_ == 0:
                            tgt = ctxT_h[half][0:64, p,
                                              qc * 512:(qc + 1) * 512]
                        else:
                            if "t" not in box:
                                box["t"] = ctxu.tile([64, 1024], fp16,
                                                     tag="cn", bufs=2,
                                                     name=f"cn_{b}")
                            tgt = box["t"][0:64, qc * 512:(qc + 1) * 512]
                        nc.vector.tensor_mul(
                            tgt, ctxUs[hp_][0:64, qc * 512:(qc + 1) * 512],
                            bcs[hp_][0:64, qc * 512:(qc + 1) * 512])
                        if hp_ == 1 and qc == 1:
                            nc.sync.dma_start(
                                ctxT_h[half][64:128, p, :], box["t"][0:64, :])

                    norm_q.extend([lambda j=j: step(j) for j in range(4)])

                sc_cur = emit_scores(0)
                prev = None  # (t-1, ets)
                for t in range(NSTEP):
                    ets = emit_exp(t, sc_cur)
                    if prev is not None:
                        emit_ctx(prev[0], prev[1])
                        if prev[0] % KT == KT - 1:
                            emit_drain(prev[0] // KT)
                    if t < NSTEP - 1:
                        sc_cur = emit_scores(t + 1)
                    prev = (t, ets)
                    if norm_q and 2 <= (t % KT) <= 5:
                        norm_q.pop(0)()
                emit_ctx(prev[0], prev[1])
                emit_drain(3)

                # ---- output projection tail ----
                for st_fn in norm_q:
                    st_fn()
                for qt in range(16):
                    half, c = qt // 8, qt % 8
                    tag = "a" if qt % 2 == 0 else "ctx"
                    po = ps.tile([128, 1024], f32, tag=tag, name=f"po_{qt}")
                    for p_ in range(2):
                        for ec in range(2):
                            nc.tensor.matmul(
                                po[:, ec * 512:(ec + 1) * 512],
                                lhsT=ctxT_h[half][:, p_,
                                                  c * 128:(c + 1) * 128],
                                rhs=Wo_sb[:, p_, ec * 512:(ec + 1) * 512],
                                start=(p_ == 0), stop=(p_ == 1))
                    ob = outsb.tile([128, 1024], fp16, tag="ob",
                                    name=f"ob_{qt}")
                    if qt % 2 == 0:
                        nc.vector.tensor_copy(ob[:], po[:])
                    else:
                        nc.scalar.copy(ob[:], po[:])
                    nc.sync.dma_start(out_d[qt * 128:(qt + 1) * 128, :],
                                      ob[:])

    nc.compile()
    return nc


def get_program(KT=9):
    key = ("nc", KT)
    if key not in _cache:
        _cache[key] = _build_program(KT)
    return _cache[key]


def make_in_maps(query, mask, W_qkv, b_qkv, W_out, b_out):
    query = np.asarray(query, dtype=np.float32)
    mask = np.asarray(mask)
    W_qkv = np.asarray(W_qkv, dtype=np.float32)
    b_qkv = np.asarray(b_qkv, dtype=np.float32)
    W_out = np.asarray(W_out, dtype=np.float32)
    bf = np.float16

    W3 = W_qkv.reshape(DIM, N_HEADS, DIM_PER_HEAD, 3)
    b3 = b_qkv.reshape(N_HEADS, DIM_PER_HEAD, 3)
    m2 = np.asarray(mask)[:, 0, :]  # [32, 2048] True = masked
    KT = max(1, int(np.ceil((~m2).sum(axis=1).max() / 128)))
    SK = KT * 128

    in_maps = []
    for c in range(N_CORES):
        b = c // 4
        h0 = (c % 4) * HEADS_PER_CORE
        hs = slice(h0, h0 + HEADS_PER_CORE)
        Wq_c = np.ascontiguousarray(
            W3[:, hs, :, 0].reshape(DIM, 256) / SCALE).astype(bf)
        Wk_c = np.ascontiguousarray(W3[:, hs, :, 1].reshape(DIM, 256)).astype(bf)
        Wv_c = np.ascontiguousarray(W3[:, hs, :, 2].reshape(DIM, 256)).astype(bf)
        bq_c = (b3[hs, :, 0].reshape(256) / SCALE).astype(np.float32)
        bk_c = b3[hs, :, 1].reshape(256).astype(np.float32)
        bvT_c = np.ascontiguousarray(b3[hs, :, 2].T).astype(np.float32)  # [64, 4]
        bqk_c = np.ascontiguousarray(
            np.stack([bq_c[:128], bq_c[128:], bk_c[:128], bk_c[128:]], axis=1))
        Wo_c = np.ascontiguousarray(
            W_out[h0 * 64:(h0 + 4) * 64, :]).astype(bf)
        xT_c = np.ascontiguousarray(query[b].T).astype(bf)

        xg_c = np.zeros((4, DIM, SK), dtype=bf)
        maskT_c = np.zeros((128, 4 * KT), dtype=np.float32)
        for hl in range(4):
            bh = b * N_HEADS + h0 + hl
            idx = np.nonzero(~m2[bh])[0]
            n = len(idx)
            idx_pad = np.zeros(SK, dtype=np.int64)
            idx_pad[:n] = idx
            xg_c[hl] = xT_c[:, idx_pad]
            padded = np.arange(SK) >= n  # [SK] True = padding slot
            maskT_c[:, hl::4] = np.where(
                padded.reshape(KT, 128).T, np.float32(-30000.0),
                np.float32(0.0))
        in_maps.append({
            "xT": xT_c, "xg": xg_c, "Wq": Wq_c, "Wk": Wk_c, "Wv": Wv_c,
            "Wo": Wo_c, "bqk": bqk_c, "bvT": bvT_c, "maskT": maskT_c,
        })
    return in_maps, KT


def gather_outputs(results, b_out):
    b_out = np.asarray(b_out, dtype=np.float32)
    out = np.zeros((B, S, DIM), dtype=np.float32)
    for c in range(N_CORES):
        out[c // 4] += results[c]["out"].astype(np.float32)
    out += b_out[None, None, :]
    return out


def kernel(query, mask, W_qkv, b_qkv, W_out, b_out):
    from concourse.bass_utils import run_bass_kernel_spmd

    in_maps, KT = make_in_maps(query, mask, W_qkv, b_qkv, W_out, b_out)
    nc = get_program(KT)
    res = run_bass_kernel_spmd(nc, in_maps, list(range(N_CORES)))
    return gather_outputs(res.results, b_out)
